# revision 18
# baseline (speedup 1.0000x reference)
"""fp8 decode-attention kernel v3 for 8 Trainium2 NeuronCores.

Head-parallel: core c owns heads [4c, 4c+4) for all 16 requests (sorted by
length desc). All heavy operands (K/V cache, weights, x) are fp8e4/bf16.

v3 over v2 (179us -> target ~100us):
- x ships pre-transposed from host: xt (bf16, for RMSNorm sum-of-squares)
  and xnt (fp8, x*ln_w in DR projection layout) - drops 32 PE transposes,
  the [16,4096] f32 x/x^2 tiles, and 32 scalar muls.
- RMSNorm partition shift via two tiny matmuls (rq ones-matmul -> [16,1]
  PSUM) instead of activation-accum over [16,4096].
- Exact-sized per-request K/V tiles (no pool cycling): K/V dma_starts are
  never gated on compute, so the DMA queues stream at full rate.
- DMA split across 3 pure queues (sync/vector/gpsimd) in need-order with
  greedy byte balancing; scalar carries only wv + splices/zshift so its
  sequencer stays free for exp. wo lands last, just before o_proj.
- extract() rebuilt: DVE tensor_scalar_mul (acc*invz -> fp8) + 4 one-hot
  matmuls per request into a bitcast [128,64] f32 PSUM strip (attn_s
  layout), replacing the serialized 64x (PE transpose -> DVE col copy)
  ping-pong (~33us -> ~8us).
- qtm masked-variant fill via 2 stride-33 diagonal DVE copies (was 64
  column copies).
"""

import sys
import types
import ctypes
import contextlib

import numpy as np
import ml_dtypes

FP8_NP = ml_dtypes.float8_e4m3
BF16_NP = ml_dtypes.bfloat16

# ---------------------------------------------------------------------------
# axon NTFF profile hook (same as baseline kernel.py)
# ---------------------------------------------------------------------------


def _install_ntff_hook():
    if "antenv.axon_hooks" in sys.modules:
        return
    try:
        lib = ctypes.CDLL("/opt/axon/libaxon_pjrt.so")
        lib.axon_start_nrt_profile.argtypes = [
            ctypes.POINTER(ctypes.c_int64),
            ctypes.c_size_t,
        ]
        lib.axon_start_nrt_profile.restype = ctypes.c_int64
        lib.axon_stop_nrt_profile.argtypes = [ctypes.c_char_p]
        lib.axon_stop_nrt_profile.restype = ctypes.c_int64
    except OSError:
        lib = None

    @contextlib.contextmanager
    def _hook(output_dir, device_ids):
        import time

        import jax

        jax.devices()

        def _start():
            if device_ids:
                ids = (ctypes.c_int64 * len(device_ids))(*device_ids)
                return lib.axon_start_nrt_profile(ids, len(device_ids))
            return lib.axon_start_nrt_profile(None, 0)

        rc = _start()
        n_try = 0
        while rc != 0 and n_try < 10:
            # a stale profile session (crashed run) blocks new ones until
            # it expires terminal-side; nudge with stop + backoff
            n_try += 1
            lib.axon_stop_nrt_profile(str(output_dir).encode())
            time.sleep(min(30, 3 * n_try))
            rc = _start()
        if rc != 0:
            raise RuntimeError(f"axon_start_nrt_profile rc={rc}")
        try:
            yield
        finally:
            n = lib.axon_stop_nrt_profile(str(output_dir).encode())
            print(f"ntff profile: {n} file(s) -> {output_dir}", file=sys.stderr)

    mod = types.ModuleType("antenv.axon_hooks")
    mod.get_axon_ntff_profile_hook = (lambda: _hook) if lib is not None else (lambda: None)
    mod.set_axon_ntff_profile_hook = lambda h: None
    sys.modules["antenv.axon_hooks"] = mod


_install_ntff_hook()

import concourse.bass as bass
import concourse.mybir as mybir
import concourse.tile as tile
from concourse.vector_clock import ScopedClock
from concourse.masks import make_identity
from concourse.bass_utils import run_bass_kernel_spmd

# ---------------------------------------------------------------------------
# walrus ">1 sem wait" workaround (same as baseline kernel.py)
# ---------------------------------------------------------------------------
_MAXW = 1


def _patched_drain_and_barrier(self, tick_clock, wait_clock):
    nc = self.nc
    probe = nc.sync.nop(nofuse=True)
    wait_clock.add_sem_waits(probe.ins, ScopedClock({None: tick_clock.global_clock}))
    si = probe.ins.sync_info
    waits = list(si.on_wait) if si is not None else []
    if len(waits) > _MAXW:
        si.on_wait = waits[:_MAXW]
        for i in range(_MAXW, len(waits), _MAXW):
            nop = nc.sync.nop(nofuse=True)
            nop.ins.sync_info = mybir.SyncInfo(
                on_wait=waits[i : i + _MAXW], on_update=[]
            )
    nc.sync.drain()
    nc.all_engine_barrier()
    assert self.sems is not None
    popped = nc._tile_sem_poison_stack.pop()
    assert popped is self._sem_poison
    nc.clear_and_free_semaphores(list(self.sems.allocated().values()))
    nc.all_engine_barrier()


tile.TileContext._drain_and_barrier = _patched_drain_and_barrier

_wsplit_counter = [0]


def _split_excess_waits(nc):
    for fn in nc.m.functions:
        for bb in fn.blocks:
            out = []
            changed = False
            for inst in bb.instructions:
                si = inst.sync_info
                if (
                    si is not None
                    and len(si.on_wait) > 1
                    and not isinstance(inst, mybir.InstAllEngineBarrier)
                ):
                    waits = list(si.on_wait)
                    for w in waits[:-1]:
                        _wsplit_counter[0] += 1
                        out.append(
                            mybir.InstNoOp(
                                name=f"I-wsplit-{_wsplit_counter[0]}",
                                engine=inst.engine,
                                sync_info=mybir.SyncInfo(
                                    on_wait=[w], on_update=[]
                                ),
                            )
                        )
                    si.on_wait = [waits[-1]]
                    changed = True
                out.append(inst)
            if changed:
                bb.instructions[:] = out


# ---------------------------------------------------------------------------

F32 = mybir.dt.float32
BF16 = mybir.dt.bfloat16
F8 = mybir.dt.float8e4
DR = mybir.MatmulPerfMode.DoubleRow
EXP = mybir.ActivationFunctionType.Exp
SQRT = mybir.ActivationFunctionType.Sqrt

P = 128
B = 16
T = 2048
D = 4096
H = 32
HD = 128
NHL = 4
NCORES = 8
EPS = 1e-6
SCALE = 1.0 / float(np.sqrt(HD))
EBIAS = -5.0       # exp bias: p8 = exp(SCALE*s - 5) keeps fp8 in range
CP = 32.0          # attn scale for fp8, folded into 1/Z at extraction
MASKV = -240.0     # fp8 mask value: exp(SCALE*(s-240)-5) ~ e^-18
NK2 = 16           # kc-pair chunks for DoubleRow projections
NTB = T // P       # 16 t-blocks
STAGE = 3          # bisect switch: 1=scores+exp, 2=+v-pass/extract, 3=full


def _build(Ls):
    """Per-core Bass kernel. Ls: 16 request lengths sorted descending."""
    nblk = [l // P + 1 for l in Ls]       # t-blocks incl. the new token
    rr_ = [l % P for l in Ls]             # new-token row within tail block
    # request-halves (8 requests each); region rows 4*(b%8)+h
    hnch = [max(Ls[8 * hf : 8 * hf + 8]) // 512 + 1 for hf in range(2)]
    hnblk = [max(nblk[8 * hf : 8 * hf + 8]) for hf in range(2)]

    nc = bass.Bass()
    xt_d = nc.dram_tensor("xt", [P, B * 32], BF16, kind="ExternalInput")
    xnt_d = nc.dram_tensor("xnt", [P, NK2 * 2 * B], F8, kind="ExternalInput")
    wq_d = nc.dram_tensor("wq", [P, NK2 * 1024], F8, kind="ExternalInput")
    wk_d = nc.dram_tensor("wk", [P, NK2 * 1024], F8, kind="ExternalInput")
    wv_d = nc.dram_tensor("wv", [P, NK2 * 1024], F8, kind="ExternalInput")
    wo_d = nc.dram_tensor("wo", [P, NHL * D], F8, kind="ExternalInput")
    mk_d = nc.dram_tensor("mk", [P, 1024], F8, kind="ExternalInput")
    ktc_d = nc.dram_tensor("ktc", [B, NHL, HD, T], F8, kind="ExternalInput")
    vc_d = nc.dram_tensor("vc", [B, P, NTB, 512], F8, kind="ExternalInput")
    out_d = nc.dram_tensor("out", [B, D], F32, kind="ExternalOutput")

    with tile.TileContext(nc) as tc:
        with (
            tc.tile_pool(name="const", bufs=1) as cst,
            tc.tile_pool(name="persist", bufs=1) as per,
            tc.tile_pool(name="wpool", bufs=2) as wp,
            tc.tile_pool(name="kpool", bufs=1) as kp,
            tc.tile_pool(name="vpool", bufs=1) as vp,
            tc.tile_pool(name="aepool", bufs=4) as aep,
            tc.tile_pool(name="outp", bufs=2) as op_,
            tc.tile_pool(name="scp", bufs=1, space="PSUM") as scp,
            tc.tile_pool(name="accp", bufs=3, space="PSUM") as accp,
            tc.tile_pool(name="ptbp", bufs=1, space="PSUM") as ptbp,
        ):
            # ---------------- SBUF tiles ----------------
            id8 = cst.tile([P, P], F8, tag="id8")
            ebias = cst.tile([P, 1], F32, tag="ebias")
            epsb = cst.tile([P, 1], F32, tag="epsb")
            ones_sb = cst.tile([P, 1], F32, tag="ones")
            xt_sb = cst.tile([P, B, 32], BF16, tag="xt")
            sq = cst.tile([P, B, 32], BF16, tag="sq")
            rq = cst.tile([P, B], F32, tag="rq")
            stdt = cst.tile([B, 1], F32, tag="std")
            rstd = cst.tile([B, 1], F32, tag="rstd")
            mk_sb = cst.tile([P, 2, 512], F8, tag="mk")
            xnt = cst.tile([P, NK2, 2, B], F8, tag="xnt")

            qt = per.tile([P, 64], F8, tag="qt")       # col 4b+h
            kt_sb = per.tile([P, 64], F8, tag="kt")    # col 4b+h
            # masked-q variants, padded to 65 so the stride-33 diagonal
            # view [:, 1024:2080] stays in-bounds
            qtm = per.tile([P, 65, 32], F8, tag="qtm")
            v_sb = per.tile([B, 512], F8, tag="vsb")
            q_nat = per.tile([B, 512], F8, tag="qnat")
            k_nat = per.tile([B, 512], F8, tag="knat")
            p8 = per.tile([P, T], F8, tag="p8")        # rows 0-31
            pT = per.tile([P, 2, NTB, 32], F8, tag="pT")
            zparts = per.tile([P, 4], F32, tag="zp")
            zsum = per.tile([P, 1], F32, tag="z")
            zsum2 = per.tile([P, 1], F32, tag="z2")
            invz = per.tile([P, 1], F32, tag="invz")   # CP/Z at region rows
            invzq = [
                per.tile([B, 1], F32, tag=f"invzq{q}", name=f"invzq{q}")
                for q in range(4)
            ]
            attn_s = per.tile([P, 64], F8, tag="attns")

            # DR moving-operand group stride (= t-extent here) must be
            # even on hw (odd strides wedge the PE) - pad to 8
            ktiles = {}
            for b in range(B):
                if Ls[b] > 0:
                    lk = (Ls[b] + 7) & ~7
                    ktiles[b] = kp.tile(
                        [P, NHL, lk], F8, tag=f"kt{b}", name=f"k{b}"
                    )
            vtiles = {}
            for b in range(B):
                vtiles[b] = vp.tile(
                    [P, nblk[b], 512], F8, tag=f"vt{b}", name=f"v{b}"
                )

            wq_t = wp.tile([P, NK2, 2, 512], F8, tag="w", name="wq")
            wk_t = wp.tile([P, NK2, 2, 512], F8, tag="w", name="wk")
            wv_t = wp.tile([P, NK2, 2, 512], F8, tag="w", name="wv")
            wo_t = wp.tile([P, NHL, D], F8, tag="w", name="wo")

            # ---------------- PSUM ----------------
            # 4 score banks: region (hf, c) -> bank c, rows [0, 32)
            scb = [
                scp.tile([P, 512], F32, tag=f"sc{c}", name=f"sc{c}")
                for c in range(4)
            ]
            # one fp8 bank, subdivided by bytes:
            #   [0:1024)    pt transposes (k=2 interleaved)
            #   [1024:1280) 8 fp8 slots for transpose4
            #   [1280:1536) attn_sT: [128, 64] f32 strip (one-hot extracts)
            #   [1536:1540) ssq: [16, 1] f32 (RMSNorm shift matmul)
            ptbank = ptbp.tile([P, 2048], F8, tag="ptbank")
            pt_ps = ptbank[:, 0:1024].rearrange("p (j v k) -> p j v k", j=NTB, k=2)
            attn_sT = ptbank[:, 1280:1536].bitcast(F32)
            attn_sTv = attn_sT.rearrange("p (hp kt b) -> p hp kt b", hp=2, b=B)
            ssq_ps = ptbank[:, 1536:1540].bitcast(F32)
            _slot = [0]

            def f8slot():
                i = _slot[0] % 8
                _slot[0] += 1
                return ptbank[:, 1024 + 32 * i : 1024 + 32 * (i + 1)].rearrange(
                    "p (b k) -> p b k", k=2
                )

            # ---------------- DMA plan ----------------
            # Need-ordered greedy byte-balance across 3 pure-DMA queues.
            # scalar only carries wv (+ splices/zshift later). wo + out on
            # sync (its sequencer never computes, so dep stalls are free).
            kbytes = [4 * l for l in Ls]
            vbytes = [nb * 512 for nb in nblk]

            def k_load(b, eng):
                if Ls[b] > 0:
                    eng.dma_start(
                        ktiles[b][:, :, 0 : Ls[b]],
                        ktc_d[b, :, :, 0 : Ls[b]].rearrange("h d t -> d h t"),
                    )

            def v_load(b, eng):
                eng.dma_start(vtiles[b][:, :, :], vc_d[b, :, 0 : nblk[b], :])

            def w_load(w_t, w_dram, eng, nch=4):
                wv_ = w_dram[:, :].rearrange(
                    "p (k kt n) -> p k kt n", kt=2, n=512
                )
                step = NK2 // nch
                for g in range(nch):
                    eng.dma_start(
                        w_t[:, step * g : step * (g + 1), :, :],
                        wv_[:, step * g : step * (g + 1)],
                    )

            # need-order work items: (kind, idx, per-partition bytes).
            # Weight CHUNKS ride the same greedy stream so no single queue's
            # head is hogged by a 2MB weight (that pushed K0-7 completion to
            # ~70us and the whole back half off the end of the DMA window).
            wv_chunks = {"wq": (wq_t, wq_d), "wk": (wk_t, wk_d), "wv": (wv_t, wv_d)}
            items = [("wq", g, 4096) for g in range(4)]
            items += [("k", b, kbytes[b]) for b in range(4)]
            items += [("wk", g, 4096) for g in range(4)]
            items += [("k", b, kbytes[b]) for b in range(4, 16)]
            items += [("wv", g, 4096) for g in range(4)]
            items += [("v", b, vbytes[b]) for b in range(16)]

            def dispatch(kind, idx, eng):
                if kind == "k":
                    k_load(idx, eng)
                elif kind == "v":
                    v_load(idx, eng)
                else:
                    w_t, w_dram = wv_chunks[kind]
                    wvw = w_dram[:, :].rearrange(
                        "p (k kt n) -> p k kt n", kt=2, n=512
                    )
                    eng.dma_start(
                        w_t[:, 4 * idx : 4 * (idx + 1), :, :],
                        wvw[:, 4 * idx : 4 * (idx + 1)],
                    )

            # Only SP (sync), Activation (scalar) and Pool (gpsimd) can
            # issue DMAs. sync is pre-seeded with the small head loads.
            qnames = ["sync", "gpsimd", "scalar"]
            qload = {"sync": 600.0, "gpsimd": 0.0, "scalar": 0.0}
            qitems = {q: [] for q in qnames}
            for kind, idx, nbytes in items:
                qmin = min(qnames, key=lambda q: qload[q])
                qitems[qmin].append((kind, idx))
                qload[qmin] += float(nbytes)

            # sync: small heads, then its share, wo last (WAR on wk buf:
            # waits k-proj; harmless - sync's seq never computes), out later
            nc.sync.dma_start(xt_sb[:], xt_d[:, :].rearrange("p (b k) -> p b k", b=B))
            nc.sync.dma_start(
                xnt[:], xnt_d[:, :].rearrange("p (k kt b) -> p k kt b", kt=2, b=B)
            )
            nc.sync.dma_start(
                mk_sb[:], mk_d[:, :].rearrange("p (t c) -> p t c", t=2)
            )
            for kind, idx in qitems["sync"]:
                dispatch(kind, idx, nc.sync)
            wo_v = wo_d[:, :].rearrange("p (h n) -> p h n", h=NHL)
            for g in range(4):
                nc.sync.dma_start(
                    wo_t[:, :, 1024 * g : 1024 * (g + 1)],
                    wo_v[:, :, 1024 * g : 1024 * (g + 1)],
                )

            # gpsimd: 3 issues, identity, then the rest
            gp = qitems["gpsimd"]
            for kind, idx in gp[:3]:
                dispatch(kind, idx, nc.gpsimd)
            make_identity(nc, id8[:])
            for kind, idx in gp[3:]:
                dispatch(kind, idx, nc.gpsimd)

            # scalar: the RMSNorm Square MUST precede any scalar dma issue
            # (a weight chunk's WAR wait would otherwise deadlock against
            # the PE -> DVE -> sq chain)
            nc.scalar.activation(
                sq[:], xt_sb[:], mybir.ActivationFunctionType.Square
            )
            for kind, idx in qitems["scalar"]:
                dispatch(kind, idx, nc.scalar)

            # ---------------- RMSNorm (xt bf16) ----------------
            # sq = xt^2 (emitted above, before scalar's DMA issues);
            # rq[p,b] = sum_kc sq ; ssq[b] = ones-matmul over p;
            # rstd = 1/sqrt(ssq/D + eps)
            nc.vector.reduce_sum(
                rq[:].rearrange("p (b o) -> p b o", o=1),
                sq[:],
                mybir.AxisListType.X,
            )
            nc.vector.memset(ones_sb[:], 1.0)
            nc.vector.memset(epsb[:], EPS)
            nc.tensor.matmul(
                ssq_ps[0:B, :], rq[:, :], ones_sb[:, :],
                start=True, stop=True, skip_group_check=True,
            )
            nc.scalar.activation(
                stdt[:], ssq_ps[0:B, :], SQRT, bias=epsb[0:B, :], scale=1.0 / D
            )
            nc.vector.reciprocal(rstd[:], stdt[:])
            nc.vector.memset(ebias[:], EBIAS)

            # ---------------- projections ----------------
            def proj(w_t, acc_ps):
                for k2 in range(NK2):
                    nc.tensor.matmul(
                        acc_ps[:],
                        xnt[:, k2, :, :],
                        w_t[:, k2, :, :],
                        start=(k2 == 0), stop=(k2 == NK2 - 1),
                        perf_mode=DR, skip_group_check=True,
                    )

            def transpose4(nat_sb, dst):
                for h in range(NHL):
                    t_ps = f8slot()
                    nc.tensor.transpose(
                        t_ps[:, :, 0], nat_sb[:, h * HD : (h + 1) * HD],
                        id8[:B, :B],
                    )
                    dv = dst[:].rearrange("p (b h) -> p b h", h=NHL)[:, :, h]
                    nc.scalar.copy(dv, t_ps[:, :, 0])

            # PE: mask matmuls open the score regions (start=True)
            def masks(hf):
                for c in range(hnch[hf]):
                    r = hf * 4 + c
                    nc.tensor.matmul(
                        scb[c][0:32, :],
                        id8[:, (r % 4) * 32 : (r % 4) * 32 + 32],
                        mk_sb[:, r // 4, :],
                        start=True, stop=False, skip_group_check=True,
                    )

            def scores(q):
                # DoubleRow head-pairing: masked q variants zero the cross
                # terms, so both heads' score rows land at 0.5 cyc/col.
                for bb in range(4):
                    b = 4 * q + bb
                    L = Ls[b]
                    for h in range(0, NHL, 2):
                        g = 4 * b + h
                        for c in range((L + 511) // 512):
                            n = min(512, L - c * 512)
                            if n > 0:
                                nc.tensor.matmul(
                                    scb[c][0:32, 0:n],
                                    qtm[:, g : g + 2, :],
                                    ktiles[b][:, h : h + 2, c * 512 : c * 512 + n],
                                    start=False, stop=False,
                                    skip_group_check=True, perf_mode=DR,
                                )

            ktv = kt_sb[:].rearrange("p (a o) -> p a o", o=1)

            def self_scores(q):
                for bb in range(4):
                    b = 4 * q + bb
                    L = Ls[b]
                    c = L // 512
                    col = L % 512
                    for h in range(0, NHL, 2):
                        g = 4 * b + h
                        nc.tensor.matmul(
                            scb[c][0:32, col : col + 1],
                            qtm[:, g : g + 2, :],
                            ktv[:, g : g + 2, :],
                            start=False, stop=False,
                            skip_group_check=True, perf_mode=DR,
                        )

            def exp_half(hf):
                for c in range(hnch[hf]):
                    nc.scalar.activation(
                        p8[0:32, c * 512 : (c + 1) * 512],
                        scb[c][0:32, :],
                        EXP, bias=ebias[0:32, :], scale=SCALE,
                        accum_out=zparts[0:32, c : c + 1],
                    )

            def zinv_half(hf):
                nc.vector.reduce_sum(
                    zsum[0:32, :], zparts[0:32, 0 : hnch[hf]],
                    mybir.AxisListType.X,
                )
                nc.vector.tensor_scalar_mul(
                    zsum2[0:32, :], zsum[0:32, :], 1.0 / CP
                )
                nc.vector.reciprocal(invz[0:32, :], zsum2[0:32, :])

            def zshift(q):
                # partition shift via SBUF->SBUF DMA: invzq[q] <- invz rows
                r0 = 16 * (q % 2)
                nc.scalar.dma_start(invzq[q][:, :], invz[r0 : r0 + 16, :])

            def splices(q):
                for bb in range(4):
                    b = 4 * q + bb
                    nc.scalar.dma_start(
                        vtiles[b][rr_[b] : rr_[b] + 1, nblk[b] - 1, :],
                        v_sb[b : b + 1, :],
                    )

            def pt_half(hf):
                for j in range(hnblk[hf]):
                    nc.tensor.transpose(
                        pt_ps[:, j, :, 0],
                        p8[0:32, j * P : (j + 1) * P],
                        id8[0:32, 0:32],
                    )

            def pt_evac(hf):
                nc.vector.tensor_copy(
                    pT[:, hf, 0 : hnblk[hf], :], pt_ps[:, 0 : hnblk[hf], :, 0]
                )

            accs = {}

            def v_quad(q):
                hf = q // 2
                co = 16 * (q % 2)
                for bb in range(4):
                    b = 4 * q + bb
                    acc = accp.tile([64, 512], F32, tag="acc", name=f"acc{b}")
                    accs[b] = acc
                    nb = nblk[b]
                    npair = nb // 2
                    for jp in range(npair):
                        nc.tensor.matmul(
                            acc[0:16, :],
                            pT[:, hf, 2 * jp : 2 * jp + 2, co : co + 16],
                            vtiles[b][:, 2 * jp : 2 * jp + 2, :],
                            start=(jp == 0), stop=(jp == npair - 1 and nb % 2 == 0),
                            perf_mode=DR, skip_group_check=True,
                        )
                    if nb % 2 == 1:
                        j = nb - 1
                        nc.tensor.matmul(
                            acc[0:16, :],
                            pT[:, hf, j, co : co + 16],
                            vtiles[b][:, j, :],
                            start=(nb == 1), stop=True, skip_group_check=True,
                        )

            def extract_mul(b):
                # DVE: acc rows (rv,h) * CP/Z -> fp8 SBUF
                q = b // 4
                ae = aep.tile([B, 512], F8, tag="ae", name=f"ae{b}")
                nc.vector.tensor_scalar_mul(
                    ae[:], accs[b][0:B, :], invzq[q][:]
                )
                return ae

            def extract_onehot(b, ae):
                # PE: 4 one-hot matmuls pull row 4rv+h of each head slice
                # into attn_sT col (h//2)*32 + (h%2)*16 + b
                rv = b % 4
                for h in range(NHL):
                    col = (h // 2) * 32 + (h % 2) * B + b
                    nc.tensor.matmul(
                        attn_sT[:, col : col + 1],
                        ae[:, h * HD : (h + 1) * HD],
                        id8[0:B, 4 * rv + h : 4 * rv + h + 1],
                        start=True, stop=True, skip_group_check=True,
                    )

            def attn_copy(hf):
                # attn_sT (f32 PSUM) -> attn_s (fp8 SBUF), this half's cols
                src = attn_sTv[:, :, :, 8 * hf : 8 * hf + 8]
                dst = attn_s[:].rearrange("p (hp kt b) -> p hp kt b", hp=2, b=B)[
                    :, :, :, 8 * hf : 8 * hf + 8
                ]
                nc.vector.tensor_copy(dst, src)

            # ---------------- main pipeline ----------------
            # PE program order drives the schedule; other engines follow.
            masks(0)
            q_ps = accp.tile([64, 512], F32, tag="acc", name="qps")
            proj(wq_t, q_ps[0:B, :])
            nc.scalar.mul(q_nat[:], q_ps[0:B, :], rstd[:])
            transpose4(q_nat, qt)

            # DVE: build masked-q variants with two stride-33 diagonal copies
            nc.vector.memset(qtm[:], 0.0)
            qtm_flat = qtm[:].rearrange("p a b -> p (a b)")
            for gh in range(2):
                diag = qtm_flat[:, 1024 * gh : 1024 * gh + 1056].rearrange(
                    "p (g r) -> p g r", r=33
                )[:, :, 0]
                nc.vector.tensor_copy(diag, qt[:, 32 * gh : 32 * gh + 32])

            # warm the Exp activation table off the critical path (the
            # table load is ~1.3us and otherwise lands right at exp(0))
            nc.scalar.activation(zsum[0:1, :], ebias[0:1, :], EXP)

            if STAGE >= 0.6:
                # scores before k-proj: stream K tiles as they land
                scores(0)
                scores(1)

            k_ps = accp.tile([64, 512], F32, tag="acc", name="kps")
            proj(wk_t, k_ps[0:B, :])
            nc.scalar.mul(k_nat[:], k_ps[0:B, :], rstd[:])
            transpose4(k_nat, kt_sb)
            if STAGE >= 0.6:
                self_scores(0)
                self_scores(1)

            if STAGE >= 1:
                exp_half(0)
                zinv_half(0)
                zshift(0)
                zshift(1)

                masks(1)
                pt_half(0)
                pt_evac(0)

                self_scores(2)
                self_scores(3)
                scores(2)
                scores(3)

                exp_half(1)
                zinv_half(1)
                zshift(2)
                zshift(3)

                v_ps = accp.tile([64, 512], F32, tag="acc", name="vps")
                proj(wv_t, v_ps[0:B, :])
                nc.scalar.mul(v_sb[:], v_ps[0:B, :], rstd[:])

            if STAGE >= 2:
                splices(0)
                splices(1)

                pt_half(1)
                pt_evac(1)

                v_quad(0)
                for b_ in range(0, 4):
                    extract_onehot(b_, extract_mul(b_))
                v_quad(1)
                for b_ in range(4, 8):
                    extract_onehot(b_, extract_mul(b_))
                attn_copy(0)

                splices(2)
                splices(3)

                v_quad(2)
                for b_ in range(8, 12):
                    extract_onehot(b_, extract_mul(b_))
                v_quad(3)
                for b_ in range(12, 16):
                    extract_onehot(b_, extract_mul(b_))
                attn_copy(1)

            if STAGE >= 3:
                # ---------------- o_proj ----------------
                # attn_s col = hp*32 + kt*16 + b (head-pair major)
                attn_v = attn_s[:].rearrange(
                    "p (hp kt b) -> p hp kt b", hp=2, kt=2
                )
                for ch in range(8):
                    o_ps = accp.tile([64, 512], F32, tag="acc", name=f"o{ch}")
                    for hp in range(2):
                        nc.tensor.matmul(
                            o_ps[0:B, :],
                            attn_v[:, hp],
                            wo_t[:, 2 * hp : 2 * hp + 2, ch * 512 : (ch + 1) * 512],
                            start=(hp == 0), stop=(hp == 1),
                            perf_mode=DR, skip_group_check=True,
                        )
                    o_sb = op_.tile([B, 512], F32, tag="osb", name=f"os{ch}")
                    nc.scalar.activation(
                        o_sb[:], o_ps[0:B, :], mybir.ActivationFunctionType.Copy,
                        scale=1.0 / CP,
                    )
                    nc.sync.dma_start(out_d[:, ch * 512 : (ch + 1) * 512], o_sb[:])
            else:
                # bisect dump -> out
                o_sb = op_.tile([B, 512], F32, tag="osb", name="osdump")
                if STAGE == 2:
                    nc.scalar.copy(o_sb[:, 0:64], attn_s[0:B, :])
                elif STAGE == 1:
                    nc.scalar.copy(o_sb[:], p8[0:B, 0:512])
                elif STAGE == 0.6:
                    nc.scalar.copy(o_sb[:], scb[0][0:B, :])
                else:
                    nc.scalar.copy(o_sb[:, 0:128], q_nat[:].bitcast(F32))
                nc.sync.dma_start(out_d[:, 0:512], o_sb[:])

    _split_excess_waits(nc)
    return nc


def _make_masks(Ls):
    """Host-built region-open masks: 2 x [128, 512] fp8.

    Region r = hf*4 + c holds rows 4*(b%8)+h of tile r//4 at col block
    (r%4)*32. Row value: 0 for cols < n_v, MASKV beyond, with
    n_v = clamp(L_b + 1 - c*512, 0, 512) (the +1 covers the self token).
    """
    mk = np.zeros((2, P, 512), np.float32)
    for hf in range(2):
        for c in range(4):
            r = hf * 4 + c
            for b in range(8 * hf, 8 * hf + 8):
                nv = min(max(Ls[b] + 1 - c * 512, 0), 512)
                for h in range(NHL):
                    v = 4 * (b % 8) + h
                    mk[r // 4, (r % 4) * 32 + v, nv:] = MASKV
    # [2, 128, 512] -> [128, 2, 512] -> [128, 1024]
    return np.ascontiguousarray(
        mk.transpose(1, 0, 2).reshape(P, 1024)
    ).astype(FP8_NP)


def _prep_inputs(x, ln_w, Wq, Wk, Wv, Wo, K_cache, V_cache, cache_lens):
    x = np.asarray(x, np.float32).reshape(B, D)
    ln_w = np.asarray(ln_w, np.float32)
    cache_lens = np.asarray(cache_lens, np.int32)
    perm = np.argsort(-cache_lens, kind="stable")
    Ls = [int(cache_lens[p]) for p in perm]
    x_s = np.ascontiguousarray(x[perm])
    mk = _make_masks(Ls)
    # xt[p, b, kc] = x_s[b, kc*128+p]  (bf16, RMSNorm input)
    xt = np.ascontiguousarray(
        x_s.reshape(B, 32, P).transpose(2, 0, 1).reshape(P, B * 32)
    ).astype(BF16_NP)
    # xnt[p, k2, kt, b] = (x*lnw)[b, k2*256+kt*128+p]  (fp8, DR layout)
    xln = x_s * ln_w[None, :]
    xnt = np.ascontiguousarray(
        xln.reshape(B, NK2, 2, P).transpose(3, 1, 2, 0).reshape(P, NK2 * 2 * B)
    ).astype(FP8_NP)
    K4 = np.asarray(K_cache, np.float32).reshape(B, T, H, HD)[perm]
    V4 = np.asarray(V_cache, np.float32).reshape(B, T, H, HD)[perm]
    in_maps = []
    for c in range(NCORES):
        h0 = c * NHL
        wq = np.ascontiguousarray(
            np.asarray(Wq, np.float32)[:, h0 * HD : (h0 + NHL) * HD]
            .reshape(NK2, 2, P, 512).transpose(2, 0, 1, 3).reshape(P, NK2 * 1024)
        ).astype(FP8_NP)
        wk = np.ascontiguousarray(
            np.asarray(Wk, np.float32)[:, h0 * HD : (h0 + NHL) * HD]
            .reshape(NK2, 2, P, 512).transpose(2, 0, 1, 3).reshape(P, NK2 * 1024)
        ).astype(FP8_NP)
        wv = np.ascontiguousarray(
            np.asarray(Wv, np.float32)[:, h0 * HD : (h0 + NHL) * HD]
            .reshape(NK2, 2, P, 512).transpose(2, 0, 1, 3).reshape(P, NK2 * 1024)
        ).astype(FP8_NP)
        wo = np.ascontiguousarray(
            np.asarray(Wo, np.float32)[h0 * HD : (h0 + NHL) * HD, :]
            .reshape(NHL, P, D).transpose(1, 0, 2).reshape(P, NHL * D)
        ).astype(FP8_NP)
        ktc = np.ascontiguousarray(
            K4[:, :, h0 : h0 + NHL, :].transpose(0, 2, 3, 1)
        ).astype(FP8_NP)
        vc = np.ascontiguousarray(
            V4[:, :, h0 : h0 + NHL, :]
            .reshape(B, NTB, P, NHL * HD).transpose(0, 2, 1, 3)
        ).astype(FP8_NP)
        in_maps.append(
            {
                "xt": xt, "xnt": xnt, "mk": mk,
                "wq": wq, "wk": wk, "wv": wv, "wo": wo,
                "ktc": ktc, "vc": vc,
            }
        )
    return in_maps, Ls, perm, x_s


def _run(x, ln_w, Wq, Wk, Wv, Wo, K_cache, V_cache, cache_lens, trace=False):
    in_maps, Ls, perm, x_s = _prep_inputs(
        x, ln_w, Wq, Wk, Wv, Wo, K_cache, V_cache, cache_lens
    )
    nc = _build(Ls)
    last_exc = None
    for _attempt in range(3):
        try:
            res = run_bass_kernel_spmd(
                nc, in_maps, core_ids=list(range(NCORES)), trace=trace
            )
            break
        except Exception as e:  # noqa: BLE001
            last_exc = e
            import time as _time

            _time.sleep(2.0)
    else:
        raise last_exc
    partial = np.zeros((B, D), np.float32)
    for c in range(NCORES):
        partial += res.results[c]["out"]
    out_sorted = x_s + partial
    out = np.empty((B, D), np.float32)
    out[perm] = out_sorted
    return out.reshape(B, 1, D), res


def kernel(x, ln_w, Wq, Wk, Wv, Wo, K_cache, V_cache, cache_lens):
    out, _ = _run(x, ln_w, Wq, Wk, Wv, Wo, K_cache, V_cache, cache_lens)
    return out
